# revision 32
# baseline (speedup 1.0000x reference)
"""Trainium2 Bass kernel for nn_Attention_85581518340337.

Restormer-style channel attention:
  x (1,64,16,64,64) -> 1x1x1 conv (64->768) -> grouped 3x3x3 conv (192 groups of 4)
  -> split q,k,v (4 heads x 64 ch) -> L2 normalize over n=t*h*w -> attn = softmax(q@kT * temp)
  -> out = attn@v -> 1x1x1 proj (256->64)

Sharding: spatial over H (64 rows -> 8 cores x 8 rows, halo 1 row each side).
Per core: folded (qkv1*dwconv) dense conv for q,k in FP8 with DoubleRow perf mode
(2x PE throughput; 7 dual-band dual-tap matmuls per chunk), streamed h-major
(each h-slice's full (t,w) field in 3 t-row-aligned chunks, 84% valid columns);
the fp8 weight scale cancels in the L2 normalization. q,k evacuated to bf16,
DMA-transposed per 1024-position group (spread over SP/Act queues, overlapped
with the next conv macro) and reduced to per-head Gram matrices on PE; two 66KB
8-core AllReduces of Gram partials (pair 0's hidden under the last conv macro);
softmax (max-shift elided: logits are bounded cosines) + projection folded into
a per-head 64x64 matrix B_h; out = concat_h(B_h) @ v computed as a bf16 15-slot
dual-band conv from separate t-major buffers.
"""

import numpy as np
import ml_dtypes

import concourse.bass as bass
import concourse.mybir as mybir
import concourse.tile as tile
from concourse import bacc
from concourse.bass_utils import run_bass_kernel_spmd

F32 = mybir.dt.float32
BF16 = mybir.dt.bfloat16
FP8 = mybir.dt.float8e4
DR = mybir.MatmulPerfMode.DoubleRow

N_CORES = 8
DIM = 64
HEADS = 4
T, H, W = 16, 64, 64
HL = H // N_CORES          # 8 output h-rows per core
HLH = HL + 2               # 10 h-rows incl halo
C3H = DIM * 3 * HEADS      # 768
N_LOC = T * HL * W         # 8192 output positions per core
NT = 512                   # one t-plane of outputs (8*64)

# padded staging layout (t, h, w) = (18, 10, 68)
PT, PH, PW = T + 2, HLH, W + 4
PLANE = PH * PW            # 680
PFREE = PT * PLANE         # 12240
GUARD = 128                # fp8 buffer guard (leading) / tail
WS = 64.0                  # fp8 weight scale (cancels in L2 normalization)
# h-major fp8 streaming layout: slice = one h-row's (t, w) field, (18, 68)
SLICE = PT * PW            # 1224
QCH = [(0, 476), (476, 476), (952, 272)]  # t-row-aligned chunks of a slice

# fp8 DoubleRow pair table: 7 instructions x {A,B} x {band0,band1} taps
# tap = (dt, dh, dw); None = zero weights
QK_PAIRS = [
    (((0, 0, 0), (0, 0, 1)), ((0, 1, 0), (0, 1, 1))),
    (((1, 0, 0), (1, 0, 1)), ((1, 1, 0), (1, 1, 1))),
    (((2, 0, 0), (2, 0, 1)), ((2, 1, 0), (2, 1, 1))),
    (((0, 2, 0), (0, 2, 1)), ((1, 2, 0), (1, 2, 1))),
    (((2, 2, 0), (2, 2, 1)), ((2, 2, 2), None)),
    (((0, 0, 2), (0, 1, 2)), ((1, 0, 2), (1, 1, 2))),
    (((2, 0, 2), (2, 1, 2)), ((0, 2, 2), (1, 2, 2))),
]

_CACHE = {}


def _build(sim=False, stop_after=None):
    nc = bacc.Bacc("TRN2", target_bir_lowering=False, debug=False,
                   num_devices=1 if sim else N_CORES)

    x_d = nc.dram_tensor("x", [DIM, PFREE], BF16, kind="ExternalInput").ap()
    x8a_d = nc.dram_tensor("x8a", [128, PFREE], FP8, kind="ExternalInput").ap()
    x8bc_d = nc.dram_tensor("x8bc", [128, 2 * PFREE], FP8, kind="ExternalInput").ap()
    dwt8_d = nc.dram_tensor("dwt8", [128, 4 * 7 * 2 * 128], FP8, kind="ExternalInput").ap()
    dwtv_d = nc.dram_tensor("dwtv", [15, 2, 128, 128], BF16, kind="ExternalInput").ap()
    projt_d = nc.dram_tensor("projt", [128, 2, DIM], F32, kind="ExternalInput").ap()
    temp_d = nc.dram_tensor("temp", [HEADS], F32, kind="ExternalInput").ap()
    eye_d = nc.dram_tensor("eye", [128, 128], F32, kind="ExternalInput").ap()
    out_d = nc.dram_tensor("out", [DIM, T, HL, W], F32, kind="ExternalOutput").ap()

    with tile.TileContext(nc) as tc:
        _emit(nc, tc, x_d, x8a_d, x8bc_d, dwt8_d, dwtv_d, projt_d, temp_d, eye_d,
              out_d, sim=sim, stop_after=stop_after)
    nc.compile()
    return nc


def _emit(nc, tc, x_d, x8a_d, x8bc_d, dwt8_d, dwtv_d, projt_d, temp_d, eye_d, out_d,
          sim=False, stop_after=None):
    import contextlib
    ctx = contextlib.ExitStack()
    with ctx:
        singles = ctx.enter_context(tc.tile_pool(name="singles", bufs=1))
        dense_p = ctx.enter_context(tc.tile_pool(name="dense", bufs=1))
        ct_p = ctx.enter_context(tc.tile_pool(name="ctp", bufs=5))
        small_p = ctx.enter_context(tc.tile_pool(name="small", bufs=2))
        out_p = ctx.enter_context(tc.tile_pool(name="outp", bufs=3))
        ps_conv = ctx.enter_context(tc.tile_pool(name="ps_conv", bufs=4, space="PSUM"))
        ps_gram = ctx.enter_context(tc.tile_pool(name="ps_gram", bufs=1, space="PSUM"))
        ps_b = ps_gram
        ps_fo = ctx.enter_context(tc.tile_pool(name="ps_fo", bufs=2, space="PSUM"))
        dram = ctx.enter_context(tc.tile_pool(name="dram", bufs=1, space="DRAM"))

        # ---- fp8 staging: pre-banded padded buffers with leading guard ----
        x8a = singles.tile([128, GUARD + PFREE + GUARD], FP8)
        x8bc = singles.tile([128, GUARD + 2 * PFREE], FP8)
        dwt8 = singles.tile([128, 4, 7, 2, 128], FP8)
        nc.sync.dma_start(out=dwt8[:].rearrange("p a b c d -> p (a b c d)"), in_=dwt8_d)
        nc.sync.dma_start(out=x8a[:, GUARD:GUARD + PFREE], in_=x8a_d)
        nc.sync.dma_start(out=x8bc[:, GUARD:], in_=x8bc_d)
        nc.gpsimd.memset(x8a[:, 0:GUARD], 0.0)
        nc.gpsimd.memset(x8a[:, GUARD + PFREE:], 0.0)
        nc.gpsimd.memset(x8bc[:, 0:GUARD], 0.0)

        # ---- bf16 staging for the v-conv: xa (w-shift band), xb (h-shift band) ----
        xa = singles.tile([128, PT, PH, PW], BF16)
        xb = singles.tile([128, PT, PH, PW], BF16)
        xaf = xa[:].rearrange("p t h w -> p (t h w)")
        xbf = xb[:].rearrange("p t h w -> p (t h w)")
        nc.gpsimd.dma_start(out=xaf[0:64, :], in_=x_d)
        nc.gpsimd.dma_start(out=xaf[64:128, 0:PFREE - 1], in_=x_d[:, 1:])
        nc.gpsimd.dma_start(out=xbf[0:64, :], in_=x_d)
        nc.gpsimd.dma_start(out=xbf[64:128, 0:PFREE - PW], in_=x_d[:, PW:])
        nc.gpsimd.memset(xaf[64:128, PFREE - 1:], 0.0)
        nc.gpsimd.memset(xbf[64:128, PFREE - PW:], 0.0)

        projt_sb = singles.tile([128, 2, DIM], F32)
        eye_sb = singles.tile([128, 128], F32)
        tsc = singles.tile([128, 2], F32)
        dwtv_sb = singles.tile([128, 15, 2, 128], BF16)

        # dense bf16 buffers for q,k (to transpose for the gram), split per
        # 1024-position group for fine-grained transpose dependencies
        NG = N_LOC // 1024
        qkg = [[dense_p.tile([128, 1024], BF16, tag=f"qk{m}_{g}", name=f"qk{m}_{g}")
                for g in range(NG)] for m in range(4)]

        gq_ps = [None, None]

        arbuf = singles.tile([128, 2, 130], F32)
        ssqk = singles.tile([128, 2, 16], F32)

        # macro order: q0, k0, q1, k1 (qkv ch-macros 0,2,1,3); v folded through attn
        macro_order = [0, 2, 1, 3]

        XA0 = GUARD
        XB0 = GUARD
        XC0 = GUARD + PFREE

        def qk_pair_table(hh):
            return [
                (x8a, XA0 + hh * SLICE - 69, PW),
                (x8a, XA0 + (hh + 1) * SLICE - 69, PW),
                (x8a, XA0 + (hh + 2) * SLICE - 69, PW),
                (x8a, XA0 + hh * SLICE + 67, SLICE),
                (x8a, XA0 + (hh + 2) * SLICE + 67, 2),
                (x8bc, XB0 + hh * SLICE - 67, SLICE),
                (x8bc, XB0 + (hh + 2) * SLICE - 67, (XC0 - XB0) - 2 * SLICE + 2 * PW),
            ]

        # per chunk: (psum col offset of first valid row, rows, dst offset)
        QEV = [(70, 6, 0), (2, 7, 384), (2, 3, 832)]

        def conv_macro(mac, after_slice=None):
            """FP8 DoubleRow folded conv for one 128-channel q/k macro tile,
            h-major streaming: per h-slice, 3 t-row-aligned chunks of the (t,w)
            field, 7 dual-band dual-tap matmuls each."""
            for hh in range(8):
                table = qk_pair_table(hh)
                gt = qkg[mac][hh]
                for ci, (cst, clen) in enumerate(QCH):
                    ps = ps_conv.tile([128, 512], F32, tag="cps", name=f"ps{mac}_{hh}_{ci}")
                    for pi, (buf, aoff, delta) in enumerate(table):
                        pstride = buf[:].ap[0][0]
                        rhs = bass.AP(tensor=buf.tensor, offset=aoff + cst,
                                      ap=[[pstride, 128], [delta, 2], [1, clen]])
                        nc.tensor.matmul(ps[:, 0:clen], dwt8[:, mac, pi], rhs,
                                         start=(pi == 0), stop=(pi == 6),
                                         perf_mode=DR)
                    soff, nr, doff = QEV[ci]
                    src = bass.AP(tensor=ps.tensor, offset=soff,
                                  ap=[[512, 128], [PW, nr], [1, W]])
                    dst = gt[:, doff:doff + nr * W].rearrange("p (r w) -> p r w", w=W)
                    if ci == 1:
                        nc.scalar.copy(out=dst, in_=src)
                    else:
                        nc.vector.tensor_copy(out=dst, in_=src)
                if mac >= 2:
                    # ssq_k from the evacuated bf16 slice: one half fused
                    # square+accum on Act, the other mul+reduce on DVE
                    sqo = small_p.tile([128, 512], F32, tag="sqo")
                    nc.scalar.activation(
                        out=sqo[:], in_=gt[:, 0:512],
                        func=mybir.ActivationFunctionType.Square,
                        accum_out=ssqk[:, mac - 2, 2 * hh:2 * hh + 1])
                    scr = small_p.tile([128, 512], F32, tag="ttr")
                    nc.vector.tensor_mul(scr[:], gt[:, 512:1024], gt[:, 512:1024])
                    nc.vector.tensor_reduce(
                        out=ssqk[:, mac - 2, 2 * hh + 1:2 * hh + 2],
                        in_=scr[:], axis=mybir.AxisListType.X,
                        op=mybir.AluOpType.add)
                if after_slice is not None:
                    after_slice(hh)

        def gram_setup(p):
            gq_ps[p] = ps_gram.tile([128, 256], F32, tag="gq", name=f"gq{p}")

        ct_tiles = {}

        def gram_tr(p, g):
            """Issue DMA-transposes for 1024 positions (group g)."""
            ct = ct_p.tile([128, 2, 8, 128], BF16, tag="ct", name=f"ct{p}_{g}")
            ct_tiles[(p, g)] = ct
            eng = nc.sync if g % 2 == 0 else nc.scalar
            eng.dma_start(out=ct[:, 0], in_=qkg[p][g][:], transpose=True)
            eng.dma_start(out=ct[:, 1], in_=qkg[2 + p][g][:], transpose=True)

        def gram_mm(p, g):
            ct = ct_tiles.pop((p, g))
            for j in range(8):
                jj = g * 8 + j
                nc.tensor.matmul(gq_ps[p][:], ct[:, 0, j, :], ct[:, :, j, :],
                                 start=(jj == 0), stop=(jj == N_LOC // 128 - 1))

        def extract_pair(p):
            """S block + diagonals of pair p into arbuf[:, p, :]."""
            nc.vector.tensor_copy(out=arbuf[:, p, 0:128], in_=gq_ps[p][:, 128:256])
            scr = small_p.tile([128, 128], F32, tag="scr")
            nc.vector.tensor_mul(scr[:], gq_ps[p][:, 0:128], eye_sb[:])
            nc.vector.tensor_reduce(out=arbuf[:, p, 128:129], in_=scr[:],
                                    axis=mybir.AxisListType.X, op=mybir.AluOpType.add)
            nc.vector.tensor_reduce(out=arbuf[:, p, 129:130], in_=ssqk[:, p, :],
                                    axis=mybir.AxisListType.X, op=mybir.AluOpType.add)

        # ---- conv phase with gram interleaved ----
        ar_in = [dram.tile([128, 130], F32, name=f"ar_in{p}") for p in range(2)]
        ar_out = [dram.tile([128, 130], F32, name=f"ar_out{p}") for p in range(2)]
        gar = singles.tile([128, 2, 130], F32)

        def launch_ar(p):
            nc.sync.dma_start(out=ar_in[p][:], in_=arbuf[:, p, :])
            if sim:
                nc.sync.dma_start(out=ar_out[p][:], in_=ar_in[p][:])
            else:
                nc.gpsimd.collective_compute(
                    "AllReduce", mybir.AluOpType.add,
                    replica_groups=[list(range(N_CORES))],
                    ins=[ar_in[p].opt()], outs=[ar_out[p].opt()])
            nc.sync.dma_start(out=gar[:, p, :], in_=ar_out[p][:])

        # ---- per-pair normalization/softmax/B tiles ----
        rno = singles.tile([128, 2, 2], F32)
        rqs = singles.tile([128, 2], F32)
        rk_d = [dram.tile([128, 1], F32, name=f"rk_d{p}") for p in range(2)]
        rkb = singles.tile([128, 2, 128], F32)
        bt_sb = [singles.tile([128, DIM], BF16, tag=f"bt{p}", name=f"bt{p}") for p in range(2)]

        def pair_chain(p):
            nc.scalar.activation(out=rno[:, p, :], in_=gar[:, p, 128:130],
                                 func=mybir.ActivationFunctionType.Sqrt)
            nc.vector.reciprocal(out=rno[:, p, :], in_=rno[:, p, :])
            nc.vector.tensor_mul(rqs[:, p:p + 1], rno[:, p, 0:1], tsc[:, p:p + 1])
            nc.sync.dma_start(out=rk_d[p][:], in_=rno[:, p, 1:2])
            src = bass.AP(tensor=rk_d[p].tensor, offset=rk_d[p].offset,
                          ap=[[0, 128], [1, 128]])
            nc.sync.dma_start(out=rkb[:, p, :], in_=src)

            lg = small_p.tile([128, 128], F32, tag="lg")
            nc.vector.tensor_mul(lg[:], gar[:, p, 0:128], rkb[:, p, :])
            nc.vector.tensor_scalar_mul(lg[:], lg[:], rqs[:, p:p + 1])
            btp = ps_b.tile([128, DIM], F32, tag="gk", name=f"btp{p}")
            at = small_p.tile([128, 64], F32, tag="at")
            sm = small_p.tile([128, 1], F32, tag="sm")
            for hf in range(2):
                hs = slice(hf * 64, (hf + 1) * 64)
                sub = lg[hs, hf * 64:(hf + 1) * 64]
                # logits are normalized cosines * temp: bounded, no max-shift needed
                nc.scalar.activation(out=at[hs], in_=sub,
                                     func=mybir.ActivationFunctionType.Exp,
                                     scale=1.0)
                nc.vector.tensor_reduce(out=sm[hs], in_=at[hs], axis=mybir.AxisListType.X,
                                        op=mybir.AluOpType.add)
                nc.vector.reciprocal(out=sm[hs], in_=sm[hs])
                nc.vector.tensor_scalar_mul(at[hs], at[hs], sm[hs])
                # B_h^T = attn_h^T @ projT_h  (partitions hf*64.. aligned throughout)
                nc.tensor.matmul(btp[hs, :], at[hs], projt_sb[hs, p, :],
                                 start=True, stop=True)
            nc.vector.tensor_copy(out=bt_sb[p][:], in_=btp[:])

        do_gram = stop_after != "convonly"
        for mi, mac in enumerate(macro_order):
            if mi >= 2 and do_gram:
                p = mi - 2
                gram_setup(p)

                def ap_cb(hh, p=p):
                    # transposes (SP/Act queues only) spread across the conv
                    # slices; gram matmuls run as one PE block afterwards
                    gram_tr(p, hh)
                conv_macro(mac, after_slice=ap_cb)
                for g in range(NG):
                    gram_mm(p, g)
                extract_pair(p)
                if stop_after != "gram":
                    launch_ar(p)
                    if p == 0:
                        pair_chain(0)
            else:
                conv_macro(mac)
            if mi == 1:
                # deferred input loads (needed only post-conv / at compose)
                nc.sync.dma_start(out=dwtv_sb[:], in_=dwtv_d.rearrange("s p k m -> k s p m"))
                nc.sync.dma_start(out=projt_sb[:], in_=projt_d)
                nc.sync.dma_start(out=eye_sb[:], in_=eye_d)
                for p_ in range(2):
                    for hf_ in range(2):
                        src_ = bass.AP(tensor=temp_d.tensor, offset=2 * p_ + hf_,
                                       ap=[[0, 64], [1, 1]])
                        nc.sync.dma_start(out=tsc[hf_ * 64:(hf_ + 1) * 64, p_:p_ + 1], in_=src_)

        if stop_after == "gram":
            nc.gpsimd.dma_start(out=out_d[:, 0, 0, :], in_=arbuf[0:64, 0, 0:64])
            return
        if stop_after in ("conv", "convonly"):
            nc.gpsimd.dma_start(out=out_d[:, 0],
                                in_=qkg[0][0][0:64, 0:NT].rearrange("p (h w) -> p h w", h=HL))
            return
        pair_chain(1)

        if stop_after == "softmax":
            return
        # ---- compose G = (B o Mfold_v): per slot GT[(band,c), e] ----
        gv = singles.tile([128, 15, DIM], BF16)
        for slot in range(15):
            gts = ps_gram.tile([128, DIM], F32, tag=("gq" if slot % 2 == 0 else "gk"),
                               name=f"gts{slot}")
            nc.tensor.matmul(gts[:], dwtv_sb[:, slot, 0, :], bt_sb[0][:],
                             start=True, stop=False)
            nc.tensor.matmul(gts[:], dwtv_sb[:, slot, 1, :], bt_sb[1][:],
                             start=False, stop=True)
            nc.vector.tensor_copy(out=gv[:, slot, :], in_=gts[:])

        # ---- v-conv: out = G * x ----
        def vslot_rhs(t, slot):
            if slot < 9:
                dti, dhi = slot // 3, slot % 3
                return xa[:, t + dti, dhi:dhi + 8, 1:65]
            if slot < 12:
                return xb[:, t + (slot - 9), 0:8, 3:67]
            return xb[:, t + (slot - 12), 2:10, 3:67]

        for t in range(T):
            fo = ps_fo.tile([64, NT], F32, tag="fo")
            for slot in range(15):
                nc.tensor.matmul(fo[:], gv[:, slot, :], vslot_rhs(t, slot),
                                 start=(slot == 0), stop=(slot == 14))
            ot = out_p.tile([64, NT], F32, tag="ot")
            if t % 2 == 0:
                nc.vector.tensor_copy(out=ot[:], in_=fo[:])
            else:
                nc.scalar.copy(out=ot[:], in_=fo[:])
            nc.sync.dma_start(out=out_d[:, t], in_=ot[:].rearrange("p (h w) -> p h w", h=HL))


def _prep_inputs(x, qkv_w, dw_w, proj_w, temperature):
    """Host-side sharding + weight layout."""
    b, c, t, h, w = x.shape
    w1 = qkv_w.reshape(C3H, DIM).astype(np.float64)   # (768, 64)
    dw = dw_w.reshape(C3H, 4, 3, 3, 3).astype(np.float64)
    # folded conv: M[o, c, dti, dhi, dwi] = sum_j dw[o, j, taps] * w1[4*(o//4)+j, c]
    j_idx = (np.arange(C3H) // 4) * 4
    w1g = w1[j_idx[:, None] + np.arange(4)[None, :], :]      # (768, 4, 64)
    mfold = np.einsum("ojtuv,ojc->octuv", dw, w1g)           # (768, 64, 3,3,3)

    # fp8 DoubleRow weights for q,k macros: dwt8[band*64+c, mac, pair, ab, o]
    dwt8 = np.zeros((128, 4, 7, 2, 128), dtype=np.float32)
    for mac in range(4):
        osl = slice(mac * 128, (mac + 1) * 128)
        for pi, pair in enumerate(QK_PAIRS):
            for ab in range(2):
                for band in range(2):
                    tap = pair[ab][band]
                    if tap is None:
                        continue
                    # h-major streaming: table's plane dim is dh, row dim is dt
                    blk = mfold[osl, :, tap[1], tap[0], tap[2]]  # (128 o, 64 c)
                    dwt8[band * 64:(band + 1) * 64, mac, pi, ab, :] = blk.T * WS
    dwt8 = dwt8.reshape(128, -1).astype(ml_dtypes.float8_e4m3)

    # v-conv slots (15): 9 xa dual-w, 3 xb dual-h (dw=2), 3 xb single (dh=2, dw=2)
    vslots = []
    for dti in range(3):
        for dhi in range(3):
            vslots.append(((dti, dhi, 0), (dti, dhi, 1)))
    for dti in range(3):
        vslots.append(((dti, 0, 2), (dti, 1, 2)))
    for dti in range(3):
        vslots.append(((dti, 2, 2), None))
    # dwtv[s, p, o, 64b + c] = mfold[512 + 128p + o, c, tap(s, b)]
    dwtv = np.zeros((15, 2, 128, 128), dtype=np.float32)
    for si, (tap0, tap1) in enumerate(vslots):
        for p in range(2):
            osl = slice(512 + p * 128, 512 + (p + 1) * 128)
            dwtv[si, p, :, 0:64] = mfold[osl, :, tap0[0], tap0[1], tap0[2]]
            if tap1 is not None:
                dwtv[si, p, :, 64:128] = mfold[osl, :, tap1[0], tap1[1], tap1[2]]
    dwtv = dwtv.astype(ml_dtypes.bfloat16)

    pw = proj_w.reshape(DIM, HEADS, DIM)              # (e, h, c)
    # projt[hf*64+c, p, e] = proj_w[e, (2p+hf)*64 + c]
    projt = np.zeros((128, 2, DIM), dtype=np.float32)
    for p in range(2):
        for hf in range(2):
            projt[hf * 64:(hf + 1) * 64, p, :] = pw[:, 2 * p + hf, :].T
    temp = np.asarray(temperature, dtype=np.float32).reshape(HEADS)
    eye = np.eye(128, dtype=np.float32)

    xp = np.zeros((c, t, h + 2, w), dtype=np.float32)
    xp[:, :, 1:h + 1, :] = x[0]
    in_maps = []
    for i in range(N_CORES):
        xs = np.zeros((c, PT, PH, PW), dtype=np.float32)
        xs[:, 1:T + 1, :, 2:W + 2] = xp[:, :, i * HL:i * HL + HLH, :]
        xsh = np.ascontiguousarray(xs.transpose(0, 2, 1, 3)).reshape(c, PFREE)
        xs = xs.reshape(c, PFREE)

        def shifted(k):
            out = np.zeros((c, PFREE), dtype=np.float32)
            out[:, 0:PFREE - k] = xsh[:, k:]
            return out

        x8a = np.concatenate([xsh, shifted(1)], axis=0).astype(ml_dtypes.float8_e4m3)
        x8b = np.concatenate([xsh, shifted(PW)], axis=0)
        x8c = np.concatenate([xsh, shifted(SLICE)], axis=0)
        x8bc = np.concatenate([x8b, x8c], axis=1).astype(ml_dtypes.float8_e4m3)
        in_maps.append({"x": xs.astype(ml_dtypes.bfloat16), "x8a": x8a,
                        "x8bc": x8bc, "dwt8": dwt8, "dwtv": dwtv, "projt": projt,
                        "temp": temp, "eye": eye})
    return in_maps


def kernel(x, qkv_w, dw_w, proj_w, temperature, _trace=False):
    if "nc" not in _CACHE:
        _CACHE["nc"] = _build()
    nc = _CACHE["nc"]
    in_maps = _prep_inputs(np.asarray(x, np.float32), np.asarray(qkv_w, np.float32),
                           np.asarray(dw_w, np.float32), np.asarray(proj_w, np.float32),
                           np.asarray(temperature, np.float32))
    kw = {}
    if _trace:
        kw = dict(trace=True, stitch_traces=True, trace_cores=list(range(N_CORES)))
    res = run_bass_kernel_spmd(nc, in_maps, core_ids=list(range(N_CORES)), **kw)
    _CACHE["last_res"] = res
    out = np.zeros((1, DIM, T, H, W), dtype=np.float32)
    for i in range(N_CORES):
        out[0, :, :, i * HL:(i + 1) * HL, :] = res.results[i]["out"]
    return out
